# revision 24
# baseline (speedup 1.0000x reference)
"""Weighted two-sided chamfer loss (AutoDecLoss) for Trainium2 -- 8 cores.

Strategy
--------
Data-parallel over the batch: core b computes the full [N=2048, M=4096]
chamfer block of batch element b; the host averages the 8 per-core scalars.

Distances come off the PE via augmented features

    d[n, m] = sum_k X[k, n] * Y[k, m],
    X = [x^2, -2x, 1] rows, Y = [1, y, y^2] rows (9 features),

computed as fp8 DoubleRow matmuls (0.5 PE cycles per output column).
Each feature is split on the host into multiple fp8 levels with per-pair
power-of-two balanced scaling (exact), giving ~12-15 bits of effective
product precision:
  forward:  e4m3 x e4m3, 3x3 levels, pairs i+j<=2  -> 54 rows (2 k-tiles)
  backward: e5m2 (rw-scaled X) x e4m3, pairs 3i+4j<=12 -> 90 rows

Min-reduction obeys the "only one PSUM operand per instruction" rule via
per-chunk chains: ACT copies the chunk's first PSUM tile to SBUF, Pool
merges later tiles into the carry (tensor_tensor min), and a DVE
tensor_tensor_reduce against the last PSUM tile emits the row-min column.
Forward chunks (4 tiles) and backward chunks (2 tiles) are interleaved
1:2 so ACT/Pool/DVE all stay ~balanced (~3 ops each per super-group).
"""

import re

import numpy as np
import ml_dtypes

import concourse.bacc as bacc
import concourse.mybir as mybir
import concourse.tile as tile
from concourse import dve_ops
from concourse.bass_utils import run_bass_kernel_spmd
from concourse.dve_spec import C0, Spec, Src0, Src1, minn
from concourse.dve_table_gen import dve_ver_for

_OP_NAME = "MIN_MIN_REDUCE_ANT"


def _ref(in0, in1, s0, s1, imm2):
    out = np.minimum(in0.astype(np.float32), in1.astype(np.float32))
    P = out.shape[0]
    body = out.reshape(P, -1)
    seed = np.asarray(s0, np.float32).reshape(-1, 1)
    acc = np.minimum(np.minimum.reduce(body, axis=-1, keepdims=True), seed)
    return out, acc


def get_min_min_reduce():
    for op in dve_ops.OPS:
        if op.name == _OP_NAME:
            return op
    spec = Spec(body=minn(Src0, Src1), accum=minn, accum_init=C0, reference=_ref)
    ver = dve_ver_for("TRN2")
    probe = dve_ops.DveOp(_OP_NAME, spec, subdim=False, uops_sha={})
    row = dve_ops._CUSTOM_DVE_ROW_BASE + len(dve_ops.OPS)
    dve_ops._SUB_OPCODE_FOR_NAME[_OP_NAME] = row
    shas = {}
    for v in ("v3", "v4"):
        try:
            probe.compile(v)
            shas[v] = probe.uops_sha.get(v)
        except ValueError as e:
            m = re.search(rf"{v}: ([0-9a-f]+)", str(e))
            if not m:
                raise
            shas[v] = m.group(1)
    op = dve_ops.DveOp(_OP_NAME, spec, subdim=False, uops_sha=shas)
    dve_ops.OPS.append(op)
    dve_ops.CUSTOM_DVE_SPECS[_OP_NAME] = spec
    assert dve_ops.get_dve_sub_opcode(_OP_NAME) == row
    assert row < 0x20
    assert ver in shas
    return op


def min_min_reduce(nc, out, in0, in1, init, accum_out):
    op = get_min_min_reduce()
    return nc.vector._custom_dve(op, out=out, in0=in0, in1=in1, s0=init,
                                 accum_out=accum_out)

B, N, M = 8, 2048, 4096
NT = N // 128          # 16
MT = M // 128          # 32
CHAMFER_EPS = 1e-6
MIN_BW = 1e-3
BIG = 3.0e38

F32 = mybir.dt.float32
BF16 = mybir.dt.bfloat16
F8E4 = mybir.dt.float8e4
F8E5 = mybir.dt.float8e5
MIN = mybir.AluOpType.min
ADD = mybir.AluOpType.add
MULT = mybir.AluOpType.mult
MAXOP = mybir.AluOpType.max
AX = mybir.AxisListType.X
DR = mybir.MatmulPerfMode.DoubleRow

E4MAX, E5MAX = 224.0, 49152.0
FWD_PAIRS = [(0, 0), (0, 1), (1, 0), (1, 1), (0, 2), (2, 0)]
BWD_PAIRS = [(i, j) for j in range(3) for i in range(5) if 3 * i + 4 * j <= 12]
KF = 9 * len(FWD_PAIRS) // 2        # 27 rows per fwd k-tile
KB = 9 * len(BWD_PAIRS) // 2        # 45 rows per bwd k-tile


def build_nc():
    nc = bacc.Bacc("TRN2", target_bir_lowering=False, debug=False, num_devices=8)
    X54 = nc.dram_tensor("X54", [KF, 2, N], F8E4, kind="ExternalInput")
    Y54 = nc.dram_tensor("Y54", [KF, 2, M], F8E4, kind="ExternalInput")
    XS90 = nc.dram_tensor("XS90", [KB, 2, N], F8E5, kind="ExternalInput")
    Y90 = nc.dram_tensor("Y90", [KB, 2, M], F8E4, kind="ExternalInput")
    outF = nc.dram_tensor("minf2", [128, 2 * NT], F32, kind="ExternalOutput")
    outB = nc.dram_tensor("minb", [128, MT], F32, kind="ExternalOutput")

    with tile.TileContext(nc) as tc:
        with (
            tc.tile_pool(name="feat", bufs=1) as fpool,
            tc.tile_pool(name="small", bufs=1) as spool,
        ):
            # DMA order = first-use order: the first fwd unit only needs sX
            # and the first half of sY, so the pipeline starts ~1.5us sooner.
            sX = fpool.tile([KF, 2, N], F8E4, tag="sX")
            nc.sync.dma_start(sX[:], X54[:])
            sY = fpool.tile([KF, 2, M], F8E4, tag="sY")
            nc.sync.dma_start(sY[:, :, 0:2048], Y54[:, :, 0:2048])
            nc.sync.dma_start(sY[:, :, 2048:M], Y54[:, :, 2048:M])
            sXS = fpool.tile([KB, 2, N], F8E5, tag="sXS")
            nc.gpsimd.dma_start(sXS[:], XS90[:])
            sYb = fpool.tile([KB, 2, M], F8E4, tag="sYb")
            nc.gpsimd.dma_start(sYb[:], Y90[:])

            minf2 = spool.tile([128, 2 * NT], F32, tag="minf2")
            nc.vector.memset(minf2[:, NT:2 * NT], BIG)
            minb = spool.tile([128, MT], F32, tag="minb")
            warm = spool.tile([1, 1], F32, tag="warm")
            nc.vector.memset(warm[:], 0.0)
            warm2 = spool.tile([1, 1], F32, tag="warm2")
            nc.scalar.copy(warm2[:], warm[:])

            with (
                tc.tile_pool(name="psum_main", bufs=1, space="PSUM") as mpool,
                tc.tile_pool(name="scratch", bufs=6) as scpool,
            ):
                arena = mpool.tile([128, 4096], F32, tag="d")

                def fill(lhsT, rhs, f0, a0, w):
                    for k in range(w // 512):
                        nc.tensor.matmul(
                            arena[:, a0 + k * 512:a0 + (k + 1) * 512], lhsT,
                            rhs[:, :, f0 + k * 512:f0 + (k + 1) * 512],
                            start=True, stop=True, perf_mode=DR)

                def unit(lhsT, rhs, f0, a0, w, acc_col):
                    """2*w cols via arena slots [a0, a0+w), [a0+w, a0+2w):
                    ACT copies the first slot, DVE min_min_reduce pairs the
                    copy with the second (single PSUM operand, one bubble)."""
                    fill(lhsT, rhs, f0, a0, w)
                    A = scpool.tile([128, 2048], F32, tag="A")
                    nc.scalar.copy(A[:, 0:w], arena[:, a0:a0 + w])
                    fill(lhsT, rhs, f0 + w, a0 + w, w)
                    tout = scpool.tile([128, 2048], F32, tag="tout")
                    min_min_reduce(nc, tout[:, 0:w], arena[:, a0 + w:a0 + 2 * w],
                                   A[:, 0:w], BIG, acc_col)

                def fwd_chunk(c):
                    lhsT = sX[:, :, c * 128:(c + 1) * 128]
                    unit(lhsT, sY, 0, 0, 2048, minf2[:, c:c + 1])

                def bwd_chunk(c):
                    lhsT = sYb[:, :, c * 128:(c + 1) * 128]
                    unit(lhsT, sXS, 0, 2048 * (c % 2), 1024, minb[:, c:c + 1])

                # all fwd units first; their accumulator tile ships out
                # while the bwd phase runs.  Epilogue math happens on host.
                for c in range(NT):
                    fwd_chunk(c)
                nc.sync.dma_start(outF[:], minf2[:])
                for c in range(MT):
                    bwd_chunk(c)
            nc.sync.dma_start(outB[:], minb[:])

    nc.compile()
    return nc


_NC_CACHE = {}


def get_nc():
    if "nc" not in _NC_CACHE:
        _NC_CACHE["nc"] = build_nc()
    return _NC_CACHE["nc"]


# ---------------- host-side fp8 feature preparation ----------------

def _q(a, dt):
    mx = E4MAX if dt is ml_dtypes.float8_e4m3 else E5MAX
    return np.clip(a, -mx, mx).astype(dt).astype(np.float64)


def _scaled_split(v, levels, dt, target=16.0):
    out = []
    r = v.astype(np.float64)
    for _ in range(levels):
        mx = np.abs(r).max() or 1.0
        e = int(np.floor(np.log2(target / mx)))
        arr = _q(r * (2.0 ** e), dt)
        out.append((arr, -e))
        r = r - arr * (2.0 ** -e)
    return out


def _pair_rows(Xsplit, dtx, Ysplit, dty, pairs):
    xr, yr = [], []
    mxl = E4MAX if dtx is ml_dtypes.float8_e4m3 else E5MAX
    myl = E4MAX if dty is ml_dtypes.float8_e4m3 else E5MAX
    for (i, j) in pairs:
        ax, ex = Xsplit[i]
        ay, ey = Ysplit[j]
        tot = ex + ey
        mx = max(np.abs(ax).max(), 1e-30)
        my = max(np.abs(ay).max(), 1e-30)
        u = int(np.round((tot + np.log2(my / mx)) / 2))
        u = min(u, int(np.floor(np.log2(mxl / mx))))
        v = tot - u
        v = min(v, int(np.floor(np.log2(myl / my))))
        u2 = tot - v
        xr.append(np.clip(ax * (2.0 ** u2), -mxl, mxl).astype(dtx))
        yr.append(np.clip(ay * (2.0 ** v), -myl, myl).astype(dty))
    return np.concatenate(xr, 0), np.concatenate(yr, 0)


def _ktiles(rows):
    K, n = rows.shape
    h = K // 2
    out = np.empty((h, 2, n), rows.dtype)
    out[:, 0, :] = rows[:h]
    out[:, 1, :] = rows[h:]
    return np.ascontiguousarray(out)


def make_in_maps(points, decoded_points, decoded_weights):
    e4, e5 = ml_dtypes.float8_e4m3, ml_dtypes.float8_e5m2
    in_maps = []
    for b in range(B):
        x = np.asarray(decoded_points[b], np.float64)
        y = np.asarray(points[b], np.float64)
        w = np.asarray(decoded_weights[b], np.float32)
        rw = (1.0 / np.maximum(w, MIN_BW)).astype(np.float64)

        X9 = np.concatenate([(x * x).T, (-2.0 * x).T, np.ones((3, N))], 0)
        Y9 = np.concatenate([np.ones((3, M)), y.T, (y * y).T], 0)
        XS9 = X9 * rw[None, :]

        Xs = _scaled_split(X9, 3, e4)
        Ys = _scaled_split(Y9, 3, e4)
        XSs = _scaled_split(XS9, 5, e5)

        xf, yf = _pair_rows(Xs, e4, Ys, e4, FWD_PAIRS)
        xb, yb = _pair_rows(XSs, e5, Ys, e4, BWD_PAIRS)

        in_maps.append({"X54": _ktiles(xf), "Y54": _ktiles(yf),
                        "XS90": _ktiles(xb), "Y90": _ktiles(yb)})
    return in_maps


def kernel(points, decoded_points, decoded_weights):
    nc = get_nc()
    in_maps = make_in_maps(points, decoded_points, decoded_weights)
    res = run_bass_kernel_spmd(nc, in_maps, core_ids=list(range(B)))
    losses = []
    for b in range(B):
        w = np.asarray(decoded_weights[b], np.float32).astype(np.float64)
        mf2 = res.results[b]["minf2"].astype(np.float64)
        mb = res.results[b]["minb"].astype(np.float64)
        minf = np.minimum(mf2[:, :NT], mf2[:, NT:])      # [128, NT]
        minf_n = minf.T.reshape(-1)                       # n = c*128+p order
        fwd = (w * np.maximum(minf_n, 0)).sum() / max(w.sum(), CHAMFER_EPS)
        bwd = np.maximum(mb, 0).T.reshape(-1).mean()
        losses.append(fwd + bwd)
    return np.asarray(np.mean(losses), dtype=np.float32)


# revision 25
# speedup vs baseline: 1.1163x; 1.1163x over previous
"""Weighted two-sided chamfer loss (AutoDecLoss) for Trainium2 -- 8 cores.

Strategy
--------
Data-parallel over the batch: core b computes the full [N=2048, M=4096]
chamfer block of batch element b; the host averages the 8 per-core scalars.

Distances come off the PE via augmented features

    d[n, m] = sum_k X[k, n] * Y[k, m],
    X = [x^2, -2x, 1] rows, Y = [1, y, y^2] rows (9 features),

computed as fp8 DoubleRow matmuls (0.5 PE cycles per output column).
Each feature is split on the host into multiple fp8 levels with per-pair
power-of-two balanced scaling (exact), giving ~12-15 bits of effective
product precision:
  forward:  e4m3 x e4m3, 3x3 levels, pairs i+j<=2  -> 54 rows (2 k-tiles)
  backward: e5m2 (rw-scaled X) x e4m3, pairs 3i+4j<=12 -> 90 rows

Min-reduction obeys the "only one PSUM operand per instruction" rule via
per-chunk chains: ACT copies the chunk's first PSUM tile to SBUF, Pool
merges later tiles into the carry (tensor_tensor min), and a DVE
tensor_tensor_reduce against the last PSUM tile emits the row-min column.
Forward chunks (4 tiles) and backward chunks (2 tiles) are interleaved
1:2 so ACT/Pool/DVE all stay ~balanced (~3 ops each per super-group).
"""

import re

import numpy as np
import ml_dtypes

import concourse.bacc as bacc
import concourse.mybir as mybir
import concourse.tile as tile
from concourse import dve_ops
from concourse.bass_utils import run_bass_kernel_spmd
from concourse.dve_spec import C0, Spec, Src0, Src1, minn
from concourse.dve_table_gen import dve_ver_for

_OP_NAME = "MIN_MIN_REDUCE_ANT"


def _ref(in0, in1, s0, s1, imm2):
    out = np.minimum(in0.astype(np.float32), in1.astype(np.float32))
    P = out.shape[0]
    body = out.reshape(P, -1)
    seed = np.asarray(s0, np.float32).reshape(-1, 1)
    acc = np.minimum(np.minimum.reduce(body, axis=-1, keepdims=True), seed)
    return out, acc


def get_min_min_reduce():
    for op in dve_ops.OPS:
        if op.name == _OP_NAME:
            return op
    spec = Spec(body=minn(Src0, Src1), accum=minn, accum_init=C0, reference=_ref)
    ver = dve_ver_for("TRN2")
    probe = dve_ops.DveOp(_OP_NAME, spec, subdim=False, uops_sha={})
    row = dve_ops._CUSTOM_DVE_ROW_BASE + len(dve_ops.OPS)
    dve_ops._SUB_OPCODE_FOR_NAME[_OP_NAME] = row
    shas = {}
    for v in ("v3", "v4"):
        try:
            probe.compile(v)
            shas[v] = probe.uops_sha.get(v)
        except ValueError as e:
            m = re.search(rf"{v}: ([0-9a-f]+)", str(e))
            if not m:
                raise
            shas[v] = m.group(1)
    op = dve_ops.DveOp(_OP_NAME, spec, subdim=False, uops_sha=shas)
    dve_ops.OPS.append(op)
    dve_ops.CUSTOM_DVE_SPECS[_OP_NAME] = spec
    assert dve_ops.get_dve_sub_opcode(_OP_NAME) == row
    assert row < 0x20
    assert ver in shas
    return op


def min_min_reduce(nc, out, in0, in1, init, accum_out):
    op = get_min_min_reduce()
    return nc.vector._custom_dve(op, out=out, in0=in0, in1=in1, s0=init,
                                 accum_out=accum_out)

B, N, M = 8, 2048, 4096
NT = N // 128          # 16
MT = M // 128          # 32
CHAMFER_EPS = 1e-6
MIN_BW = 1e-3
BIG = 3.0e38

F32 = mybir.dt.float32
BF16 = mybir.dt.bfloat16
F8E4 = mybir.dt.float8e4
F8E5 = mybir.dt.float8e5
MIN = mybir.AluOpType.min
ADD = mybir.AluOpType.add
MULT = mybir.AluOpType.mult
MAXOP = mybir.AluOpType.max
AX = mybir.AxisListType.X
DR = mybir.MatmulPerfMode.DoubleRow

E4MAX, E5MAX = 224.0, 49152.0
FWD_PAIRS = [(0, 0), (0, 1), (1, 0), (1, 1), (0, 2), (2, 0)]
BWD_PAIRS = [(i, j) for j in range(3) for i in range(5) if 3 * i + 4 * j <= 12]
KF = 9 * len(FWD_PAIRS) // 2        # 27 rows per fwd k-tile
KB = 9 * len(BWD_PAIRS) // 2        # 45 rows per bwd k-tile


def build_nc():
    nc = bacc.Bacc("TRN2", target_bir_lowering=False, debug=False, num_devices=8)
    X54 = nc.dram_tensor("X54", [KF, 2, N], F8E4, kind="ExternalInput")
    Y54 = nc.dram_tensor("Y54", [KF, 2, M], F8E4, kind="ExternalInput")
    XS90 = nc.dram_tensor("XS90", [KB, 2, N], F8E5, kind="ExternalInput")
    Y90 = nc.dram_tensor("Y90", [KB, 2, M], F8E4, kind="ExternalInput")
    outF = nc.dram_tensor("minf2", [128, 2 * NT], F32, kind="ExternalOutput")
    outB = nc.dram_tensor("minb", [128, MT], F32, kind="ExternalOutput")

    with tile.TileContext(nc) as tc:
        with (
            tc.tile_pool(name="feat", bufs=1) as fpool,
            tc.tile_pool(name="small", bufs=1) as spool,
        ):
            # DMA order = first-use order: the first fwd unit only needs sX
            # and the first half of sY, so the pipeline starts ~1.5us sooner.
            sX = fpool.tile([KF, 2, N], F8E4, tag="sX")
            nc.sync.dma_start(sX[:], X54[:])
            sY = fpool.tile([KF, 2, M], F8E4, tag="sY")
            nc.sync.dma_start(sY[:, :, 0:2048], Y54[:, :, 0:2048])
            nc.sync.dma_start(sY[:, :, 2048:M], Y54[:, :, 2048:M])
            sXS = fpool.tile([KB, 2, N], F8E5, tag="sXS")
            nc.gpsimd.dma_start(sXS[:], XS90[:])
            sYb = fpool.tile([KB, 2, M], F8E4, tag="sYb")
            nc.gpsimd.dma_start(sYb[:], Y90[:])

            minf2 = spool.tile([128, 2 * NT], F32, tag="minf2")
            minb = spool.tile([128, MT], F32, tag="minb")
            warm = spool.tile([1, 1], F32, tag="warm")
            nc.vector.memset(warm[:], 0.0)
            warm2 = spool.tile([1, 1], F32, tag="warm2")
            nc.scalar.copy(warm2[:], warm[:])

            with (
                tc.tile_pool(name="psum_main", bufs=4, space="PSUM") as mpool,
                tc.tile_pool(name="scratch", bufs=6) as scpool,
            ):
                def mmtile(lhsT, rhs, f0):
                    """Fill one [128,1024] PSUM tile with 2 DR matmuls."""
                    ps = mpool.tile([128, 1024], F32, tag="d")
                    for k in range(2):
                        nc.tensor.matmul(
                            ps[:, k * 512:(k + 1) * 512], lhsT,
                            rhs[:, :, f0 + k * 512:f0 + (k + 1) * 512],
                            start=True, stop=True, perf_mode=DR)
                    return ps

                def unit(lhsT, rhs, f0, acc_col):
                    """2048 cols: ACT copies the first PSUM tile to SBUF,
                    DVE TTR pairs the copy with the second tile (one PSUM
                    operand) and emits the row-min column."""
                    ps0 = mmtile(lhsT, rhs, f0)
                    A = scpool.tile([128, 1024], F32, tag="A")
                    nc.scalar.copy(A[:], ps0[:])
                    ps1 = mmtile(lhsT, rhs, f0 + 1024)
                    tout = scpool.tile([128, 1024], F32, tag="tout")
                    min_min_reduce(nc, tout[:], ps1[:], A[:], BIG, acc_col)

                def fwd_chunk(c):
                    lhsT = sX[:, :, c * 128:(c + 1) * 128]
                    unit(lhsT, sY, 0, minf2[:, c:c + 1])
                    unit(lhsT, sY, 2048, minf2[:, NT + c:NT + c + 1])

                def bwd_chunk(c):
                    lhsT = sYb[:, :, c * 128:(c + 1) * 128]
                    unit(lhsT, sXS, 0, minb[:, c:c + 1])

                # all fwd units first; their accumulator tile ships out
                # while the bwd phase runs.  Epilogue math happens on host.
                for c in range(NT):
                    fwd_chunk(c)
                nc.sync.dma_start(outF[:], minf2[:])
                for c in range(MT):
                    bwd_chunk(c)
            nc.sync.dma_start(outB[:], minb[:])

    nc.compile()
    return nc


_NC_CACHE = {}


def get_nc():
    if "nc" not in _NC_CACHE:
        _NC_CACHE["nc"] = build_nc()
    return _NC_CACHE["nc"]


# ---------------- host-side fp8 feature preparation ----------------

def _q(a, dt):
    mx = E4MAX if dt is ml_dtypes.float8_e4m3 else E5MAX
    return np.clip(a, -mx, mx).astype(dt).astype(np.float64)


def _scaled_split(v, levels, dt, target=16.0):
    out = []
    r = v.astype(np.float64)
    for _ in range(levels):
        mx = np.abs(r).max() or 1.0
        e = int(np.floor(np.log2(target / mx)))
        arr = _q(r * (2.0 ** e), dt)
        out.append((arr, -e))
        r = r - arr * (2.0 ** -e)
    return out


def _pair_rows(Xsplit, dtx, Ysplit, dty, pairs):
    xr, yr = [], []
    mxl = E4MAX if dtx is ml_dtypes.float8_e4m3 else E5MAX
    myl = E4MAX if dty is ml_dtypes.float8_e4m3 else E5MAX
    for (i, j) in pairs:
        ax, ex = Xsplit[i]
        ay, ey = Ysplit[j]
        tot = ex + ey
        mx = max(np.abs(ax).max(), 1e-30)
        my = max(np.abs(ay).max(), 1e-30)
        u = int(np.round((tot + np.log2(my / mx)) / 2))
        u = min(u, int(np.floor(np.log2(mxl / mx))))
        v = tot - u
        v = min(v, int(np.floor(np.log2(myl / my))))
        u2 = tot - v
        xr.append(np.clip(ax * (2.0 ** u2), -mxl, mxl).astype(dtx))
        yr.append(np.clip(ay * (2.0 ** v), -myl, myl).astype(dty))
    return np.concatenate(xr, 0), np.concatenate(yr, 0)


def _ktiles(rows):
    K, n = rows.shape
    h = K // 2
    out = np.empty((h, 2, n), rows.dtype)
    out[:, 0, :] = rows[:h]
    out[:, 1, :] = rows[h:]
    return np.ascontiguousarray(out)


def make_in_maps(points, decoded_points, decoded_weights):
    e4, e5 = ml_dtypes.float8_e4m3, ml_dtypes.float8_e5m2
    in_maps = []
    for b in range(B):
        x = np.asarray(decoded_points[b], np.float64)
        y = np.asarray(points[b], np.float64)
        w = np.asarray(decoded_weights[b], np.float32)
        rw = (1.0 / np.maximum(w, MIN_BW)).astype(np.float64)

        X9 = np.concatenate([(x * x).T, (-2.0 * x).T, np.ones((3, N))], 0)
        Y9 = np.concatenate([np.ones((3, M)), y.T, (y * y).T], 0)
        XS9 = X9 * rw[None, :]

        Xs = _scaled_split(X9, 3, e4)
        Ys = _scaled_split(Y9, 3, e4)
        XSs = _scaled_split(XS9, 5, e5)

        xf, yf = _pair_rows(Xs, e4, Ys, e4, FWD_PAIRS)
        xb, yb = _pair_rows(XSs, e5, Ys, e4, BWD_PAIRS)

        in_maps.append({"X54": _ktiles(xf), "Y54": _ktiles(yf),
                        "XS90": _ktiles(xb), "Y90": _ktiles(yb)})
    return in_maps


def kernel(points, decoded_points, decoded_weights):
    nc = get_nc()
    in_maps = make_in_maps(points, decoded_points, decoded_weights)
    res = run_bass_kernel_spmd(nc, in_maps, core_ids=list(range(B)))
    losses = []
    for b in range(B):
        w = np.asarray(decoded_weights[b], np.float32).astype(np.float64)
        mf2 = res.results[b]["minf2"].astype(np.float64)
        mb = res.results[b]["minb"].astype(np.float64)
        minf = np.minimum(mf2[:, :NT], mf2[:, NT:])      # [128, NT]
        minf_n = minf.T.reshape(-1)                       # n = c*128+p order
        fwd = (w * np.maximum(minf_n, 0)).sum() / max(w.sum(), CHAMFER_EPS)
        bwd = np.maximum(mb, 0).T.reshape(-1).mean()
        losses.append(fwd + bwd)
    return np.asarray(np.mean(losses), dtype=np.float32)
